# revision 1
# baseline (speedup 1.0000x reference)
"""Trainium2 Bass kernel for LoRACrossAttnProcessor.

Strategy:
- Host: fold LoRA (W_eff = W + up @ down, exact), pre-transpose X/E/W so all
  device matmuls contract over the partition dim with no on-chip transposes.
- Shard: data-parallel over batch, 2 batch items per core, 8 cores.
- Device (per core, all fp32r = fp32 rounded to 11-bit mantissa; fp32 PSUM):
    K.T = Wk_eff @ E.T   [1280, 154]   (both batches, N padded to 256)
    V   = E @ Wv_eff.T   [77, 1280]    (per batch, natural layout)
    Q.T = Wq_eff @ X.T   [1280, 1024]  (per batch)
    per (batch, head): scores.T = (K.T)_h.T-chunks @ (Q.T)_h  -> [77, 1024]
      exps = exp(scores.T * scale)  (ACT, fused scale)
      sumexp = ones.T @ exps (PE), recip (DVE), partition-broadcast (DMA)
      A.T_h = V_h.T @ exps via col-tiled matmuls, normalized by recip (DVE STT)
    O = A @ Wo_eff.T  [1024, 1280]  (natural layout, streamed out)
- Host: gather batches, add bo.
"""

import numpy as np
from contextlib import ExitStack

import concourse.bass as bass
import concourse.mybir as mybir
import concourse.tile as tile
from concourse import bacc
from concourse.bass_utils import run_bass_kernel_spmd

F32 = mybir.dt.float32
F32R = mybir.dt.float32r
AF = mybir.ActivationFunctionType
MULT = mybir.AluOpType.mult

H = 8
B, S, C = 16, 1024, 1280
SENC, CENC = 77, 1024
D = C // H  # 160
NCORES = 8
BPC = B // NCORES  # 2 batches per core
P = 128
NCI_Q = C // P  # 10 contraction tiles for Q/O proj
NCI_KV = CENC // P  # 8 contraction tiles for K/V proj
NCO = C // P  # 10 output-channel tiles
NST = S // 512  # 2 seq chunks of 512
EPAD = 256  # padded encoder column count (2*77 -> 256)
ATTN_SCALE = 1.0 / float(np.sqrt(D))
OCHUNKS = [(0, 512), (512, 512), (1024, 256)]


def head_chunks(h):
    """Split head h's channel range [160h, 160h+160) into PE-tile-aligned
    blocks: size in {32, 64, 128}, offset % size == 0 within a 128-tile.

    Returns [(tile, offset, size, local_d0)]."""
    out = []
    g0, g1 = D * h, D * (h + 1)
    g = g0
    while g < g1:
        t, off = divmod(g, P)
        rem = min(P - off, g1 - g)
        s = 128
        while s > rem or off % s != 0:
            s //= 2
        assert s >= 32
        out.append((t, off, s, g - g0))
        g += s
    return out


def aligned_ranges(r0, r1):
    """Decompose [r0, r1) (within one 128 tile) into blocks of size 32/64/128
    with offset % size == 0 (SBUF partition-access alignment rule)."""
    out = []
    g = r0
    while g < r1:
        s = 128
        while s > r1 - g or g % s != 0:
            s //= 2
        out.append((g, s))
        g += s
    return out


def build():
    nc = bacc.Bacc("TRN2", target_bir_lowering=False, debug=False)
    xt_d = nc.dram_tensor("xt", [BPC, C, S], F32, kind="ExternalInput")
    et_d = nc.dram_tensor("et", [CENC, EPAD], F32, kind="ExternalInput")
    wqt_d = nc.dram_tensor("wqt", [C, C], F32, kind="ExternalInput")
    wkt_d = nc.dram_tensor("wkt", [CENC, C], F32, kind="ExternalInput")
    wvt_d = nc.dram_tensor("wvt", [CENC, C], F32, kind="ExternalInput")
    wot_d = nc.dram_tensor("wot", [C, C], F32, kind="ExternalInput")
    out_d = nc.dram_tensor("out", [BPC, S, C], F32, kind="ExternalOutput")

    with tile.TileContext(nc) as tc, ExitStack() as ctx:
        big = ctx.enter_context(tc.tile_pool(name="big", bufs=3))
        wblk = ctx.enter_context(tc.tile_pool(name="wblk", bufs=2))
        raw = ctx.enter_context(tc.tile_pool(name="raw", bufs=2))
        persist = ctx.enter_context(tc.tile_pool(name="persist", bufs=1))
        expp = ctx.enter_context(tc.tile_pool(name="expp", bufs=2))
        smallp = ctx.enter_context(tc.tile_pool(name="smallp", bufs=2))
        stag = ctx.enter_context(tc.tile_pool(name="stag", bufs=2))
        psum = ctx.enter_context(tc.tile_pool(name="psum", bufs=7, space="PSUM"))

        rnd_engines = [nc.vector, nc.vector]

        # ---- constants ----
        ones77f = persist.tile([SENC, 1], F32, tag="ones77f")
        nc.vector.memset(ones77f, 1.0)
        ones77r = persist.tile([SENC, 1], F32R, tag="ones77r")
        nc.vector.tensor_copy(out=ones77r, in_=ones77f)
        zeros_f = persist.tile([P, 2 * SENC], F32, tag="zeros_f")
        nc.vector.memset(zeros_f, 0.0)

        # ---- load & round E.T  [1024, 256] -> et_r [128, 8, 256] ----
        et_r = persist.tile([P, NCI_KV, EPAD], F32R, tag="et")
        for ci in range(NCI_KV):
            rw = raw.tile([P, NCI_Q, P], F32, tag="raw")
            nc.sync.dma_start(
                out=rw[:, :2, :].rearrange("p a b -> p (a b)"),
                in_=et_d.ap()[ci * P : (ci + 1) * P, :],
            )
            rnd_engines[ci % 2].tensor_copy(
                out=et_r[:, ci, :], in_=rw[:, :2, :].rearrange("p a b -> p (a b)")
            )

        # ---- K.T projection (both batches): kt_r[t] = [128, 154] ----
        kt_r = []
        for t in range(NCO):
            blk = wblk.tile([P, NCI_Q, EPAD], F32R, tag="wblk")
            for ci in range(NCI_KV):
                rw = raw.tile([P, NCI_Q, P], F32, tag="raw")
                nc.sync.dma_start(
                    out=rw[:, 0, :],
                    in_=wkt_d.ap()[ci * P : (ci + 1) * P, t * P : (t + 1) * P],
                )
                rnd_engines[ci % 2].tensor_copy(
                    out=blk[:, ci, :P], in_=rw[:, 0, :]
                )
            ps = psum.tile([P, EPAD], F32, tag="ps")
            for ci in range(NCI_KV):
                nc.tensor.matmul(
                    ps,
                    blk[:, ci, :P],
                    et_r[:, ci, :],
                    start=(ci == 0),
                    stop=(ci == NCI_KV - 1),
                )
            # Two parity-masked K.T copies: even heads' rows in kte (odd rows
            # zero) and vice versa. Scores matmuls can then use full 128-row
            # base-0 tiles; zeros kill the other heads' contributions.
            # (Accumulating matmuls from different PE row-groups into one
            # PSUM crash at runtime, so per-head row-chunks are not usable.)
            kte = persist.tile([P, 2 * SENC], F32R, tag=f"kte{t}", name=f"kte{t}")
            kto = persist.tile([P, 2 * SENC], F32R, tag=f"kto{t}", name=f"kto{t}")
            nc.vector.tensor_copy(out=kte, in_=zeros_f[:, : 2 * SENC])
            nc.vector.tensor_copy(out=kto, in_=zeros_f[:, : 2 * SENC])
            for h in range(H):
                r0 = max(D * h, P * t)
                r1 = min(D * h + D, P * t + P)
                if r0 >= r1:
                    continue
                dst = kte if h % 2 == 0 else kto
                for o, s in aligned_ranges(r0 - P * t, r1 - P * t):
                    nc.vector.tensor_copy(
                        out=dst[o : o + s, :], in_=ps[o : o + s, : 2 * SENC]
                    )
            kt_r.append((kte, kto))

        # ---- V projection (per batch, natural layout): v_nat[b] [77, 1280] ----
        v_nat = []
        for b in range(BPC):
            v_nat.append(
                persist.tile([SENC, C], F32R, tag=f"vnat{b}", name=f"vnat{b}")
            )
        for cc in range(0, C, 256):
            blk = wblk.tile([P, NCI_Q, EPAD], F32R, tag="wblk")
            for ci in range(NCI_KV):
                rw = raw.tile([P, NCI_Q, P], F32, tag="raw")
                nc.sync.dma_start(
                    out=rw[:, :2, :].rearrange("p a b -> p (a b)"),
                    in_=wvt_d.ap()[ci * P : (ci + 1) * P, cc : cc + 256],
                )
                rnd_engines[ci % 2].tensor_copy(
                    out=blk[:, ci, :],
                    in_=rw[:, :2, :].rearrange("p a b -> p (a b)"),
                )
            for b in range(BPC):
                ps = psum.tile([SENC, 512], F32, tag="ps")
                for ci in range(NCI_KV):
                    nc.tensor.matmul(
                        ps[:, :256],
                        et_r[:, ci, b * SENC : (b + 1) * SENC],
                        blk[:, ci, :],
                        start=(ci == 0),
                        stop=(ci == NCI_KV - 1),
                    )
                nc.vector.tensor_copy(
                    out=v_nat[b][:, cc : cc + 256], in_=ps[:, :256]
                )

        # ---- load & round X.T per batch: xt_r[b] [128, 10, 1024] ----
        xt_r = [None] * BPC
        for b in range(BPC):
            xt_r[b] = big.tile([P, NCI_Q, S], F32R, tag="big", name=f"xt{b}")
            for ci in range(NCI_Q):
                rw = raw.tile([P, NCI_Q, P], F32, tag="raw")
                nc.sync.dma_start(
                    out=rw[:, :8, :].rearrange("p a b -> p (a b)"),
                    in_=xt_d.ap()[b, ci * P : (ci + 1) * P, :],
                )
                rnd_engines[ci % 2].tensor_copy(
                    out=xt_r[b][:, ci, :],
                    in_=rw[:, :8, :].rearrange("p a b -> p (a b)"),
                )

        # ---- Q.T projection, batch-major (Wq streamed per batch) ----
        qt_r = [None] * BPC
        for b in range(BPC):
            qt_r[b] = big.tile([P, NCO, S], F32R, tag="big", name=f"qt{b}")
            for co in range(NCO):
                blk = wblk.tile([P, NCI_Q, EPAD], F32R, tag="wblk")
                rwb = raw.tile([P, NCI_Q, P], F32, tag="raw")
                nc.sync.dma_start(
                    out=rwb,
                    in_=wqt_d.ap()[:, co * P : (co + 1) * P].rearrange(
                        "(ci p) c -> p ci c", p=P
                    ),
                )
                for ci in range(NCI_Q):
                    rnd_engines[ci % 2].tensor_copy(
                        out=blk[:, ci, :P], in_=rwb[:, ci, :]
                    )
                for st in range(NST):
                    ps = psum.tile([P, 512], F32, tag="ps")
                    for ci in range(NCI_Q):
                        nc.tensor.matmul(
                            ps,
                            blk[:, ci, :P],
                            xt_r[b][:, ci, st * 512 : st * 512 + 512],
                            start=(ci == 0),
                            stop=(ci == NCI_Q - 1),
                        )
                    nc.vector.tensor_copy(
                        out=qt_r[b][:, co, st * 512 : st * 512 + 512], in_=ps
                    )

        # ---- attention per (batch, head) -> at_r[b] [128, 10, 1024] ----
        at_r = [None] * BPC
        for b in range(BPC):
            at_r[b] = big.tile([P, NCO, S], F32R, tag="big", name=f"at{b}")
            for h in range(H):
                hch = head_chunks(h)
                for st in range(NST):
                    sl = slice(st * 512, st * 512 + 512)
                    # scores.T [77, 512]: full 128-row tiles of parity-masked
                    # K.T accumulated over the tiles this head touches.
                    tiles = sorted({t for (t, _, _, _) in hch})
                    ps_s = psum.tile([SENC, 512], F32, tag="ps")
                    for i, t in enumerate(tiles):
                        nc.tensor.matmul(
                            ps_s,
                            kt_r[t][h % 2][:, b * SENC : (b + 1) * SENC],
                            qt_r[b][:, t, sl],
                            start=(i == 0),
                            stop=(i == len(tiles) - 1),
                        )
                    exps = expp.tile([SENC, 512], F32R, tag="exps")
                    nc.scalar.activation(
                        out=exps, in_=ps_s, func=AF.Exp, scale=ATTN_SCALE
                    )
                    # sumexp [1, 512] on PE; reciprocal; partition-broadcast
                    ps_se = psum.tile([1, 512], F32, tag="ps")
                    nc.tensor.matmul(ps_se, ones77r, exps, start=True, stop=True)
                    rec = smallp.tile([1, 512], F32, tag="rec")
                    nc.vector.reciprocal(out=rec, in_=ps_se)
                    bc = smallp.tile([P, 512], F32, tag="bc")
                    nc.gpsimd.partition_broadcast(bc, rec)
                    # A.T_h = V_h.T @ exps, landed at global partition offsets
                    # via col-tiling; normalize by bc while copying to SBUF.
                    for t, off, size, l0 in hch:
                        ps_av = psum.tile([P, 512], F32, tag="ps")
                        nc.tensor.matmul(
                            ps_av[0:size, :],
                            v_nat[b][:, D * h + l0 : D * h + l0 + size],
                            exps,
                            start=True,
                            stop=True,
                        )
                        avt = smallp.tile([P, 512], F32R, tag="avt")
                        nc.vector.scalar_tensor_tensor(
                            out=avt[0:size, :],
                            in0=ps_av[0:size, :],
                            scalar=1.0,
                            in1=bc[0:size, :],
                            op0=MULT,
                            op1=MULT,
                        )
                        # fp32r matmuls can't target PSUM partition offsets;
                        # DMA does the partition shift into the assembled A.T.
                        nc.sync.dma_start(
                            out=at_r[b][off : off + size, t, sl],
                            in_=avt[0:size, :],
                        )

        # ---- O projection as O.T (Wo streamed once, stage-major) ----
        # O.T[co, m] = sum_ch Wo_eff[co, ch] A[m, ch]; DMA writes DRAM with a
        # transposed access pattern (partition dim -> channel, 4B stride).
        for co in range(NCO):
            blk = wblk.tile([P, NCI_Q, EPAD], F32R, tag="wblk")
            rwb = raw.tile([P, NCI_Q, P], F32, tag="raw")
            nc.sync.dma_start(
                out=rwb,
                in_=wot_d.ap()[:, co * P : (co + 1) * P].rearrange(
                    "(ci p) c -> p ci c", p=P
                ),
            )
            for ci in range(NCI_Q):
                rnd_engines[ci % 2].tensor_copy(
                    out=blk[:, ci, :P], in_=rwb[:, ci, :]
                )
            for b in range(BPC):
                for st in range(NST):
                    ps = psum.tile([P, 512], F32, tag="ps")
                    for ci in range(NCI_Q):
                        nc.tensor.matmul(
                            ps,
                            blk[:, ci, :P],
                            at_r[b][:, ci, st * 512 : st * 512 + 512],
                            start=(ci == 0),
                            stop=(ci == NCI_Q - 1),
                        )
                    ot = stag.tile([P, 512], F32, tag="ot")
                    nc.scalar.copy(out=ot, in_=ps)
                    nc.sync.dma_start(
                        out=out_d.ap()[
                            b, st * 512 : st * 512 + 512, co * P : (co + 1) * P
                        ].rearrange("s c -> c s"),
                        in_=ot,
                    )

    nc.compile()
    return nc


_NC_CACHE = []


def _get_nc():
    if not _NC_CACHE:
        _NC_CACHE.append(build())
    return _NC_CACHE[0]


def make_in_maps(hidden_states, encoder_hidden_states, Wq, Wk, Wv, Wo,
                 q_down, q_up, k_down, k_up, v_down, v_up, o_down, o_up):
    wq = (Wq.astype(np.float64) + q_up.astype(np.float64) @ q_down.astype(np.float64))
    wk = (Wk.astype(np.float64) + k_up.astype(np.float64) @ k_down.astype(np.float64))
    wv = (Wv.astype(np.float64) + v_up.astype(np.float64) @ v_down.astype(np.float64))
    wo = (Wo.astype(np.float64) + o_up.astype(np.float64) @ o_down.astype(np.float64))
    wqt = np.ascontiguousarray(wq.T.astype(np.float32))
    wkt = np.ascontiguousarray(wk.T.astype(np.float32))
    wvt = np.ascontiguousarray(wv.T.astype(np.float32))
    wot = np.ascontiguousarray(wo.T.astype(np.float32))

    in_maps = []
    for c in range(NCORES):
        hs = hidden_states[c * BPC : (c + 1) * BPC]  # [2, S, C]
        xt = np.ascontiguousarray(hs.transpose(0, 2, 1).astype(np.float32))
        enc = encoder_hidden_states[c * BPC : (c + 1) * BPC]  # [2, 77, 1024]
        et = np.zeros((CENC, EPAD), np.float32)
        for b in range(BPC):
            et[:, b * SENC : (b + 1) * SENC] = enc[b].T
        in_maps.append(
            {"xt": xt, "et": et, "wqt": wqt, "wkt": wkt, "wvt": wvt, "wot": wot}
        )
    return in_maps


def kernel(hidden_states, encoder_hidden_states, Wq, Wk, Wv, Wo, bo,
           q_down, q_up, k_down, k_up, v_down, v_up, o_down, o_up):
    nc = _get_nc()
    in_maps = make_in_maps(
        hidden_states, encoder_hidden_states, Wq, Wk, Wv, Wo,
        q_down, q_up, k_down, k_up, v_down, v_up, o_down, o_up,
    )
    res = run_bass_kernel_spmd(nc, in_maps, list(range(NCORES)))
    out = np.concatenate([res.results[c]["out"] for c in range(NCORES)], axis=0)
    out = out + bo.astype(np.float32)[None, None, :]
    return out.astype(np.float32)



# revision 3
# speedup vs baseline: 21.4030x; 21.4030x over previous
"""Trainium2 Bass kernel for LoRACrossAttnProcessor (v2, bf16).

Strategy:
- Host: fold LoRA (W_eff = W + up @ down, exact in f64), pre-transpose
  X/E/W so every device matmul contracts over the partition dim, cast
  everything to bf16 (rel err ~0.4% << 2e-2 tolerance). Wv is scattered
  into a (head, tile)-pair layout with zero padding so attention-output
  matmuls can accumulate whole 128-row PSUM tiles (no partition-shift
  DMAs).
- Shard: data-parallel over batch, 2 batch items per core, 8 cores.
- Device (per core; PSUM fp32, SBUF bf16):
    K.T tiles  = Wk_eff.T-blocks @ E.T   -> parity-masked kte/kto
    V          = E @ Wvm (pair layout)   -> vm [77, 16*128]
    Q.T        = Wq_eff.T-blocks @ X.T   -> qt [128, 10, 1024]
    per (b, st, h): scores.T = kt_h.T @ qt  [77, 512]
      exps = exp(scores.T * scale) (ACT), sumexp = ones.T @ exps (PE),
      recip (DVE), partition-broadcast (GPSIMD), expn = exps * bc (DVE)
    per (b, st, tile): A.T tile = sum_h vm_pair @ expn_h  (accumulated
      full-128-row matmuls; zeros in vm kill other heads' rows)
    O[s, c]    = A.T-blocks.T @ Wo_eff.T  (natural layout; contiguous
      2.5KB-row DMA to DRAM -- the v1 kernel's transposed output DMA
      was 2.6M 4-byte descriptors and 90% of its runtime)
- Host: gather batches, cast fp32, add bo.
"""

import numpy as np
from contextlib import ExitStack

import ml_dtypes

import concourse.bass as bass
import concourse.mybir as mybir
import concourse.tile as tile
from concourse import bacc
from concourse.bass_utils import run_bass_kernel_spmd

F32 = mybir.dt.float32
BF16 = mybir.dt.bfloat16
AF = mybir.ActivationFunctionType

H = 8
B, S, C = 16, 1024, 1280
SENC, CENC = 77, 1024
D = C // H  # 160
NCORES = 8
BPC = B // NCORES  # 2 batches per core
P = 128
NCI_Q = C // P  # 10 contraction tiles for Q/O proj
NCI_KV = CENC // P  # 8 contraction tiles for K/V proj
NCO = C // P  # 10 output-channel tiles
NST = S // 512  # 2 seq chunks of 512
SENC2 = 2 * SENC  # 154
ATTN_SCALE = 1.0 / float(np.sqrt(D))
OCHUNKS = [(0, 512), (512, 512), (1024, 256)]

# (head, tile) pairs: head h covers channels [160h, 160h+160); tile t covers
# [128t, 128t+128). Each pair gets one 128-col slot in the vm layout.
PAIRS = []
for _h in range(H):
    for _t in range(NCO):
        lo = max(D * _h, P * _t)
        hi = min(D * _h + D, P * _t + P)
        if lo < hi:
            PAIRS.append((_h, _t, lo, hi))
NPAIR = len(PAIRS)  # 16
PAIRS_OF_TILE = {t: [i for i, p in enumerate(PAIRS) if p[1] == t] for t in range(NCO)}
TILES_OF_HEAD = {h: sorted({p[1] for p in PAIRS if p[0] == h}) for h in range(H)}


def aligned_ranges(r0, r1):
    """Decompose [r0, r1) (within one 128 tile) into blocks of size 32/64/128
    with offset % size == 0 (SBUF partition-access alignment rule)."""
    out = []
    g = r0
    while g < r1:
        s = 128
        while s > r1 - g or g % s != 0:
            s //= 2
        out.append((g, s))
        g += s
    return out


def build():
    nc = bacc.Bacc("TRN2", target_bir_lowering=False, debug=False)
    xt_d = nc.dram_tensor("xt", [BPC, C, S], BF16, kind="ExternalInput")
    et_d = nc.dram_tensor("et", [CENC, SENC2], BF16, kind="ExternalInput")
    wqt_d = nc.dram_tensor("wqt", [C, C], BF16, kind="ExternalInput")
    wkt_d = nc.dram_tensor("wkt", [CENC, C], BF16, kind="ExternalInput")
    wvm_d = nc.dram_tensor("wvm", [CENC, NPAIR * P], BF16, kind="ExternalInput")
    wot_d = nc.dram_tensor("wot", [C, C], BF16, kind="ExternalInput")
    out_d = nc.dram_tensor("out", [BPC, S, C], BF16, kind="ExternalOutput")

    with tile.TileContext(nc) as tc, ExitStack() as ctx:
        big = ctx.enter_context(tc.tile_pool(name="big", bufs=4))
        wpool = ctx.enter_context(tc.tile_pool(name="wpool", bufs=2))
        persist = ctx.enter_context(tc.tile_pool(name="persist", bufs=1))
        expp = ctx.enter_context(tc.tile_pool(name="expp", bufs=3))
        expnp = ctx.enter_context(tc.tile_pool(name="expnp", bufs=2))
        smallp = ctx.enter_context(tc.tile_pool(name="smallp", bufs=3))
        stag = ctx.enter_context(tc.tile_pool(name="stag", bufs=2))
        psA = ctx.enter_context(tc.tile_pool(name="psA", bufs=3, space="PSUM"))
        psS = ctx.enter_context(tc.tile_pool(name="psS", bufs=2, space="PSUM"))
        psE = ctx.enter_context(tc.tile_pool(name="psE", bufs=1, space="PSUM"))
        psV = ctx.enter_context(tc.tile_pool(name="psV", bufs=2, space="PSUM"))

        # ---- constants ----
        ones77 = persist.tile([SENC, 1], BF16, tag="ones77")
        nc.vector.memset(ones77, 1.0)

        # ---- load E.T  [1024, 154] -> et_s [128, 8, 154] ----
        et_s = persist.tile([P, NCI_KV, SENC2], BF16, tag="et")
        nc.sync.dma_start(
            out=et_s, in_=et_d.ap().rearrange("(ci p) e -> p ci e", p=P)
        )

        # ---- K.T projection: kte/kto[t] = parity-masked [128, 154] ----
        wk_s = wpool.tile([P, NCI_KV, C], BF16, tag="w")
        nc.sync.dma_start(
            out=wk_s, in_=wkt_d.ap().rearrange("(ci p) c -> p ci c", p=P)
        )
        kt_r = []
        for t in range(NCO):
            ps = psA.tile([P, 512], F32, tag="ps")
            for ci in range(NCI_KV):
                nc.tensor.matmul(
                    ps[:, :SENC2],
                    wk_s[:, ci, t * P : (t + 1) * P],
                    et_s[:, ci, :],
                    start=(ci == 0),
                    stop=(ci == NCI_KV - 1),
                )
            kte = persist.tile([P, SENC2], BF16, tag=f"kte{t}", name=f"kte{t}")
            kto = persist.tile([P, SENC2], BF16, tag=f"kto{t}", name=f"kto{t}")
            nc.vector.memset(kte, 0.0)
            nc.vector.memset(kto, 0.0)
            for h in range(H):
                r0 = max(D * h, P * t)
                r1 = min(D * h + D, P * t + P)
                if r0 >= r1:
                    continue
                dst = kte if h % 2 == 0 else kto
                for o, sz in aligned_ranges(r0 - P * t, r1 - P * t):
                    nc.vector.tensor_copy(
                        out=dst[o : o + sz, :], in_=ps[o : o + sz, :SENC2]
                    )
            kt_r.append((kte, kto))

        # ---- V projection into (head,tile)-pair layout: vm[b] [77, 16*128] ----
        wvm_s = wpool.tile([P, NCI_KV, NPAIR * P], BF16, tag="w")
        nc.sync.dma_start(
            out=wvm_s, in_=wvm_d.ap().rearrange("(ci p) c -> p ci c", p=P)
        )
        vm = []
        for b in range(BPC):
            vm.append(persist.tile([SENC, NPAIR, P], BF16, tag=f"vm{b}", name=f"vm{b}"))
        for b in range(BPC):
            for cc in range(0, NPAIR * P, 512):
                ps = psA.tile([P, 512], F32, tag="ps")
                for ci in range(NCI_KV):
                    nc.tensor.matmul(
                        ps[:SENC, :],
                        et_s[:, ci, b * SENC : (b + 1) * SENC],
                        wvm_s[:, ci, cc : cc + 512],
                        start=(ci == 0),
                        stop=(ci == NCI_KV - 1),
                    )
                nc.vector.tensor_copy(
                    out=vm[b][:, cc // P : cc // P + 4, :].rearrange(
                        "p a b -> p (a b)"
                    ),
                    in_=ps[:SENC, :],
                )

        # ---- load X.T per batch: x_s[b] [128, 10, 1024] ----
        x_s = [None] * BPC
        for b in range(BPC):
            x_s[b] = big.tile([P, NCI_Q, S], BF16, tag="big", name=f"xt{b}")
            nc.sync.dma_start(
                out=x_s[b], in_=xt_d.ap()[b].rearrange("(ci p) s -> p ci s", p=P)
            )

        # ---- Q.T projection: qt[b] [128, 10, 1024] ----
        wq_s = wpool.tile([P, NCI_Q, C], BF16, tag="w")
        nc.sync.dma_start(
            out=wq_s, in_=wqt_d.ap().rearrange("(ci p) c -> p ci c", p=P)
        )
        qt = [None] * BPC
        for b in range(BPC):
            qt[b] = big.tile([P, NCO, S], BF16, tag="big", name=f"qt{b}")
        for co in range(NCO):
            for st in range(NST):
                sl = slice(st * 512, st * 512 + 512)
                for b in range(BPC):
                    ps = psA.tile([P, 512], F32, tag="ps")
                    for ci in range(NCI_Q):
                        nc.tensor.matmul(
                            ps,
                            wq_s[:, ci, co * P : (co + 1) * P],
                            x_s[b][:, ci, sl],
                            start=(ci == 0),
                            stop=(ci == NCI_Q - 1),
                        )
                    nc.vector.tensor_copy(out=qt[b][:, co, sl], in_=ps)

        # ---- attention -> at[b] [128, 10, 1024] (A.T, bf16) ----
        at = [None] * BPC
        for b in range(BPC):
            at[b] = big.tile([P, NCO, S], BF16, tag="big", name=f"at{b}")
        for b in range(BPC):
            bsl = slice(b * SENC, (b + 1) * SENC)
            for st in range(NST):
                sl = slice(st * 512, st * 512 + 512)
                expn = expnp.tile([SENC, H, 512], BF16, tag="expn")
                for h in range(H):
                    tiles = TILES_OF_HEAD[h]
                    ps_s = psS.tile([SENC, 512], F32, tag="ps")
                    for i, t in enumerate(tiles):
                        nc.tensor.matmul(
                            ps_s,
                            kt_r[t][h % 2][:, bsl],
                            qt[b][:, t, sl],
                            start=(i == 0),
                            stop=(i == len(tiles) - 1),
                        )
                    exps = expp.tile([SENC, 512], BF16, tag="exps")
                    nc.scalar.activation(
                        out=exps, in_=ps_s, func=AF.Exp, scale=ATTN_SCALE
                    )
                    ps_se = psE.tile([1, 512], F32, tag="ps")
                    nc.tensor.matmul(ps_se, ones77, exps, start=True, stop=True)
                    rec = smallp.tile([1, 512], F32, tag="rec")
                    nc.vector.reciprocal(out=rec, in_=ps_se)
                    bc = smallp.tile([SENC, 512], F32, tag="bc")
                    nc.gpsimd.partition_broadcast(bc, rec)
                    nc.vector.tensor_mul(
                        out=expn[:, h, :], in0=exps, in1=bc
                    )
                for t in range(NCO):
                    pairs = PAIRS_OF_TILE[t]
                    ps_av = psV.tile([P, 512], F32, tag="ps")
                    for i, pi in enumerate(pairs):
                        ph = PAIRS[pi][0]
                        nc.tensor.matmul(
                            ps_av,
                            vm[b][:, pi, :],
                            expn[:, ph, :],
                            start=(i == 0),
                            stop=(i == len(pairs) - 1),
                        )
                    nc.vector.tensor_copy(out=at[b][:, t, sl], in_=ps_av)

        # ---- O projection, natural layout: out[b, s, c] ----
        wo_s = wpool.tile([P, NCI_Q, C], BF16, tag="w")
        nc.sync.dma_start(
            out=wo_s, in_=wot_d.ap().rearrange("(ci p) c -> p ci c", p=P)
        )
        for b in range(BPC):
            for stile in range(S // P):
                s0 = stile * P
                ost = stag.tile([P, C], BF16, tag="ost")
                pso = [
                    psA.tile([P, 512], F32, tag="ps", name=f"pso{k}")
                    for k in range(len(OCHUNKS))
                ]
                for ci in range(NCI_Q):
                    for k, (c0, cn) in enumerate(OCHUNKS):
                        nc.tensor.matmul(
                            pso[k][:, :cn],
                            at[b][:, ci, s0 : s0 + P],
                            wo_s[:, ci, c0 : c0 + cn],
                            start=(ci == 0),
                            stop=(ci == NCI_Q - 1),
                        )
                for k, (c0, cn) in enumerate(OCHUNKS):
                    nc.vector.tensor_copy(
                        out=ost[:, c0 : c0 + cn], in_=pso[k][:, :cn]
                    )
                nc.sync.dma_start(out=out_d.ap()[b, s0 : s0 + P, :], in_=ost)

    nc.compile()
    return nc


_NC_CACHE = []


def _get_nc():
    if not _NC_CACHE:
        _NC_CACHE.append(build())
    return _NC_CACHE[0]


def make_in_maps(hidden_states, encoder_hidden_states, Wq, Wk, Wv, Wo,
                 q_down, q_up, k_down, k_up, v_down, v_up, o_down, o_up):
    bf = ml_dtypes.bfloat16
    wq = (Wq.astype(np.float64) + q_up.astype(np.float64) @ q_down.astype(np.float64))
    wk = (Wk.astype(np.float64) + k_up.astype(np.float64) @ k_down.astype(np.float64))
    wv = (Wv.astype(np.float64) + v_up.astype(np.float64) @ v_down.astype(np.float64))
    wo = (Wo.astype(np.float64) + o_up.astype(np.float64) @ o_down.astype(np.float64))
    wqt = np.ascontiguousarray(wq.T).astype(bf)
    wkt = np.ascontiguousarray(wk.T).astype(bf)
    wot = np.ascontiguousarray(wo.T).astype(bf)
    wvt = wv.T  # [CENC, C] f64
    wvm = np.zeros((CENC, NPAIR * P), np.float64)
    for i, (h, t, lo, hi) in enumerate(PAIRS):
        wvm[:, i * P + (lo - P * t) : i * P + (hi - P * t)] = wvt[:, lo:hi]
    wvm = wvm.astype(bf)

    in_maps = []
    for c in range(NCORES):
        hs = hidden_states[c * BPC : (c + 1) * BPC]  # [2, S, C]
        xt = np.ascontiguousarray(hs.transpose(0, 2, 1)).astype(bf)
        enc = encoder_hidden_states[c * BPC : (c + 1) * BPC]  # [2, 77, 1024]
        et = np.empty((CENC, SENC2), np.float32)
        for b in range(BPC):
            et[:, b * SENC : (b + 1) * SENC] = enc[b].T
        in_maps.append(
            {
                "xt": xt,
                "et": et.astype(bf),
                "wqt": wqt,
                "wkt": wkt,
                "wvm": wvm,
                "wot": wot,
            }
        )
    return in_maps


def kernel(hidden_states, encoder_hidden_states, Wq, Wk, Wv, Wo, bo,
           q_down, q_up, k_down, k_up, v_down, v_up, o_down, o_up):
    nc = _get_nc()
    in_maps = make_in_maps(
        hidden_states, encoder_hidden_states, Wq, Wk, Wv, Wo,
        q_down, q_up, k_down, k_up, v_down, v_up, o_down, o_up,
    )
    res = run_bass_kernel_spmd(nc, in_maps, list(range(NCORES)))
    out = np.concatenate(
        [np.asarray(res.results[c]["out"]).astype(np.float32) for c in range(NCORES)],
        axis=0,
    )
    out = out + bo.astype(np.float32)[None, None, :]
    return out.astype(np.float32)


# revision 9
# speedup vs baseline: 25.3558x; 1.1847x over previous
"""Trainium2 Bass kernel for LoRACrossAttnProcessor (v2, bf16).

Strategy:
- Host: fold LoRA (W_eff = W + up @ down, exact in f64), pre-transpose
  X/E/W so every device matmul contracts over the partition dim, cast
  everything to bf16 (rel err ~0.4% << 2e-2 tolerance). Wv is scattered
  into a (head, tile)-pair layout with zero padding so attention-output
  matmuls can accumulate whole 128-row PSUM tiles (no partition-shift
  DMAs).
- Shard: data-parallel over batch, 2 batch items per core, 8 cores.
- Device (per core; PSUM fp32, SBUF bf16):
    K.T tiles  = Wk_eff.T-blocks @ E.T   -> parity-masked kte/kto
    V          = E @ Wvm (pair layout)   -> vm [77, 16*128]
    Q.T        = Wq_eff.T-blocks @ X.T   -> qt [128, 10, 1024]
    per (b, st, h): scores.T = kt_h.T @ qt  [77, 512]
      exps = exp(scores.T * scale) (ACT), sumexp = ones.T @ exps (PE),
      recip (DVE), partition-broadcast (GPSIMD), expn = exps * bc (DVE)
    per (b, st, tile): A.T tile = sum_h vm_pair @ expn_h  (accumulated
      full-128-row matmuls; zeros in vm kill other heads' rows)
    O[s, c]    = A.T-blocks.T @ Wo_eff.T  (natural layout; contiguous
      2.5KB-row DMA to DRAM -- the v1 kernel's transposed output DMA
      was 2.6M 4-byte descriptors and 90% of its runtime)
- Host: gather batches, cast fp32, add bo.
"""

import numpy as np
from contextlib import ExitStack

import ml_dtypes

import concourse.bass as bass
import concourse.mybir as mybir
import concourse.tile as tile
from concourse import bacc
from concourse.bass_utils import run_bass_kernel_spmd

F32 = mybir.dt.float32
BF16 = mybir.dt.bfloat16
AF = mybir.ActivationFunctionType

H = 8
B, S, C = 16, 1024, 1280
SENC, CENC = 77, 1024
D = C // H  # 160
NCORES = 8
BPC = B // NCORES  # 2 batches per core
P = 128
NCI_Q = C // P  # 10 contraction tiles for Q/O proj
NCI_KV = CENC // P  # 8 contraction tiles for K/V proj
NCO = C // P  # 10 output-channel tiles
NST = S // 512  # 2 seq chunks of 512
SENC2 = 2 * SENC  # 154
ATTN_SCALE = 1.0 / float(np.sqrt(D))
OCHUNKS = [(0, 512), (512, 512), (1024, 256)]

# (head, tile) pairs: head h covers channels [160h, 160h+160); tile t covers
# [128t, 128t+128). Each pair gets one 128-col slot in the vm layout.
PAIRS = []
for _h in range(H):
    for _t in range(NCO):
        lo = max(D * _h, P * _t)
        hi = min(D * _h + D, P * _t + P)
        if lo < hi:
            PAIRS.append((_h, _t, lo, hi))
NPAIR = len(PAIRS)  # 16
PAIRS_OF_TILE = {t: [i for i, p in enumerate(PAIRS) if p[1] == t] for t in range(NCO)}
TILES_OF_HEAD = {h: sorted({p[1] for p in PAIRS if p[0] == h}) for h in range(H)}


def aligned_ranges(r0, r1):
    """Decompose [r0, r1) (within one 128 tile) into blocks of size 32/64/128
    with offset % size == 0 (SBUF partition-access alignment rule)."""
    out = []
    g = r0
    while g < r1:
        s = 128
        while s > r1 - g or g % s != 0:
            s //= 2
        out.append((g, s))
        g += s
    return out


def build():
    nc = bacc.Bacc("TRN2", target_bir_lowering=False, debug=False)
    xt_d = nc.dram_tensor("xt", [BPC, C, S], BF16, kind="ExternalInput")
    et_d = nc.dram_tensor("et", [CENC, SENC2], BF16, kind="ExternalInput")
    wqt_d = nc.dram_tensor("wqt", [C, C], BF16, kind="ExternalInput")
    wkt_d = nc.dram_tensor("wkt", [CENC, C], BF16, kind="ExternalInput")
    wvm_d = nc.dram_tensor("wvm", [CENC, NPAIR * P], BF16, kind="ExternalInput")
    wot_d = nc.dram_tensor("wot", [C, C], BF16, kind="ExternalInput")
    out_d = nc.dram_tensor("out", [BPC, S, C], BF16, kind="ExternalOutput")

    with tile.TileContext(nc) as tc, ExitStack() as ctx:
        big = ctx.enter_context(tc.tile_pool(name="big", bufs=4))
        wpool = ctx.enter_context(tc.tile_pool(name="wpool", bufs=2))
        persist = ctx.enter_context(tc.tile_pool(name="persist", bufs=1))
        expp = ctx.enter_context(tc.tile_pool(name="expp", bufs=3))
        expnp = ctx.enter_context(tc.tile_pool(name="expnp", bufs=2))
        smallp = ctx.enter_context(tc.tile_pool(name="smallp", bufs=3))
        stag = ctx.enter_context(tc.tile_pool(name="stag", bufs=2))
        psA = ctx.enter_context(tc.tile_pool(name="psA", bufs=3, space="PSUM"))
        psSE = ctx.enter_context(tc.tile_pool(name="psSE", bufs=3, space="PSUM"))
        psV = ctx.enter_context(tc.tile_pool(name="psV", bufs=2, space="PSUM"))

        # ---- constants ----
        ones77 = persist.tile([SENC, 1], BF16, tag="ones77")
        nc.vector.memset(ones77, 1.0)

        # ---- load E.T  [1024, 154] -> et_s [128, 8, 154] ----
        et_s = persist.tile([P, NCI_KV, SENC2], BF16, tag="et")
        nc.sync.dma_start(
            out=et_s, in_=et_d.ap().rearrange("(ci p) e -> p ci e", p=P)
        )

        # ---- early DMAs: X.T per batch, Wk, Wq (overlap with K-proj) ----
        x_s = [None] * BPC
        for b in range(BPC):
            x_s[b] = big.tile([P, NCI_Q, S], BF16, tag="big", name=f"xt{b}")
            nc.sync.dma_start(
                out=x_s[b], in_=xt_d.ap()[b].rearrange("(ci p) s -> p ci s", p=P)
            )
        wk_s = wpool.tile([P, NCI_KV, C], BF16, tag="w")
        nc.sync.dma_start(
            out=wk_s, in_=wkt_d.ap().rearrange("(ci p) c -> p ci c", p=P)
        )
        wq_s = wpool.tile([P, NCI_Q, C], BF16, tag="w", name="wq_s")
        nc.sync.dma_start(
            out=wq_s, in_=wqt_d.ap().rearrange("(ci p) c -> p ci c", p=P)
        )

        # ---- K.T projection: kte/kto[t] = parity-masked [128, 154] ----
        kt_r = []
        for t in range(NCO):
            ps = psA.tile([P, 512], F32, tag="ps")
            for ci in range(NCI_KV):
                nc.tensor.matmul(
                    ps[:, :SENC2],
                    wk_s[:, ci, t * P : (t + 1) * P],
                    et_s[:, ci, :],
                    start=(ci == 0),
                    stop=(ci == NCI_KV - 1),
                )
            kte = persist.tile([P, SENC2], BF16, tag=f"kte{t}", name=f"kte{t}")
            kto = persist.tile([P, SENC2], BF16, tag=f"kto{t}", name=f"kto{t}")
            nc.vector.memset(kte, 0.0)
            nc.vector.memset(kto, 0.0)
            for h in range(H):
                r0 = max(D * h, P * t)
                r1 = min(D * h + D, P * t + P)
                if r0 >= r1:
                    continue
                dst = kte if h % 2 == 0 else kto
                for o, sz in aligned_ranges(r0 - P * t, r1 - P * t):
                    nc.vector.tensor_copy(
                        out=dst[o : o + sz, :], in_=ps[o : o + sz, :SENC2]
                    )
            kt_r.append((kte, kto))

        # ---- Q.T projection: qt[b] [128, 10, 1024] ----
        qt = [None] * BPC
        for b in range(BPC):
            qt[b] = big.tile([P, NCO, S], BF16, tag="big", name=f"qt{b}")
        for co in range(NCO):
            for st in range(NST):
                sl = slice(st * 512, st * 512 + 512)
                for b in range(BPC):
                    ps = psA.tile([P, 512], F32, tag="ps")
                    for ci in range(NCI_Q):
                        nc.tensor.matmul(
                            ps,
                            wq_s[:, ci, co * P : (co + 1) * P],
                            x_s[b][:, ci, sl],
                            start=(ci == 0),
                            stop=(ci == NCI_Q - 1),
                        )
                    nc.scalar.copy(out=qt[b][:, co, sl], in_=ps)

        # ---- V projection into (head,tile)-pair layout: vm[b] [77, 16*128] ----
        wvm_s = wpool.tile([P, NCI_KV, NPAIR * P], BF16, tag="w", name="wvm_s")
        nc.sync.dma_start(
            out=wvm_s, in_=wvm_d.ap().rearrange("(ci p) c -> p ci c", p=P)
        )
        vm = []
        for b in range(BPC):
            vm.append(persist.tile([SENC, NPAIR, P], BF16, tag=f"vm{b}", name=f"vm{b}"))
        for b in range(BPC):
            for cc in range(0, NPAIR * P, 512):
                ps = psA.tile([P, 512], F32, tag="ps")
                for ci in range(NCI_KV):
                    nc.tensor.matmul(
                        ps[:SENC, :],
                        et_s[:, ci, b * SENC : (b + 1) * SENC],
                        wvm_s[:, ci, cc : cc + 512],
                        start=(ci == 0),
                        stop=(ci == NCI_KV - 1),
                    )
                nc.vector.tensor_copy(
                    out=vm[b][:, cc // P : cc // P + 4, :].rearrange(
                        "p a b -> p (a b)"
                    ),
                    in_=ps[:SENC, :],
                )

        # ---- attention -> at[b] [128, 10, 1024] (A.T, bf16) ----
        at = [None] * BPC
        for b in range(BPC):
            at[b] = big.tile([P, NCO, S], BF16, tag="big", name=f"at{b}")
        for b in range(BPC):
            bsl = slice(b * SENC, (b + 1) * SENC)
            for st in range(NST):
                sl = slice(st * 512, st * 512 + 512)
                expn = expnp.tile([SENC, H, 512], BF16, tag="expn")
                for h in range(H):
                    tiles = TILES_OF_HEAD[h]
                    ps_s = psSE.tile([SENC, 512], F32, tag="ps")
                    for i, t in enumerate(tiles):
                        nc.tensor.matmul(
                            ps_s,
                            kt_r[t][h % 2][:, bsl],
                            qt[b][:, t, sl],
                            start=(i == 0),
                            stop=(i == len(tiles) - 1),
                        )
                    exps = expp.tile([SENC, 512], BF16, tag="exps")
                    nc.scalar.activation(
                        out=exps, in_=ps_s, func=AF.Exp, scale=ATTN_SCALE
                    )
                    ps_se = psSE.tile([SENC, 512], F32, tag="ps", name="ps_se")
                    nc.tensor.matmul(
                        ps_se[0:1, :], ones77, exps, start=True, stop=True
                    )
                    rec = smallp.tile([1, 512], F32, tag="rec")
                    nc.vector.reciprocal_approx_fast(out=rec, in_=ps_se[0:1, :])
                    bc = smallp.tile([SENC, 512], F32, tag="bc")
                    nc.gpsimd.partition_broadcast(bc, rec)
                    nc.vector.tensor_mul(
                        out=expn[:, h, :], in0=exps, in1=bc
                    )
                for t in range(NCO):
                    pairs = PAIRS_OF_TILE[t]
                    ps_av = psV.tile([P, 512], F32, tag="ps")
                    for i, pi in enumerate(pairs):
                        ph = PAIRS[pi][0]
                        nc.tensor.matmul(
                            ps_av,
                            vm[b][:, pi, :],
                            expn[:, ph, :],
                            start=(i == 0),
                            stop=(i == len(pairs) - 1),
                        )
                    nc.vector.tensor_copy(out=at[b][:, t, sl], in_=ps_av)

        # ---- O projection, natural layout: out[b, s, c] ----
        wo_s = wpool.tile([P, NCI_Q, C], BF16, tag="w")
        nc.sync.dma_start(
            out=wo_s, in_=wot_d.ap().rearrange("(ci p) c -> p ci c", p=P)
        )
        for b in range(BPC):
            for stile in range(S // P):
                s0 = stile * P
                ost = stag.tile([P, C], BF16, tag="ost")
                pso = [
                    psA.tile([P, 512], F32, tag="ps", name=f"pso{k}")
                    for k in range(len(OCHUNKS))
                ]
                for ci in range(NCI_Q):
                    for k, (c0, cn) in enumerate(OCHUNKS):
                        nc.tensor.matmul(
                            pso[k][:, :cn],
                            at[b][:, ci, s0 : s0 + P],
                            wo_s[:, ci, c0 : c0 + cn],
                            start=(ci == 0),
                            stop=(ci == NCI_Q - 1),
                        )
                for k, (c0, cn) in enumerate(OCHUNKS):
                    nc.vector.tensor_copy(
                        out=ost[:, c0 : c0 + cn], in_=pso[k][:, :cn]
                    )
                nc.sync.dma_start(out=out_d.ap()[b, s0 : s0 + P, :], in_=ost)

    nc.compile()
    return nc


_NC_CACHE = []


def _get_nc():
    if not _NC_CACHE:
        _NC_CACHE.append(build())
    return _NC_CACHE[0]


def make_in_maps(hidden_states, encoder_hidden_states, Wq, Wk, Wv, Wo,
                 q_down, q_up, k_down, k_up, v_down, v_up, o_down, o_up):
    bf = ml_dtypes.bfloat16
    wq = (Wq.astype(np.float64) + q_up.astype(np.float64) @ q_down.astype(np.float64))
    wk = (Wk.astype(np.float64) + k_up.astype(np.float64) @ k_down.astype(np.float64))
    wv = (Wv.astype(np.float64) + v_up.astype(np.float64) @ v_down.astype(np.float64))
    wo = (Wo.astype(np.float64) + o_up.astype(np.float64) @ o_down.astype(np.float64))
    wqt = np.ascontiguousarray(wq.T).astype(bf)
    wkt = np.ascontiguousarray(wk.T).astype(bf)
    wot = np.ascontiguousarray(wo.T).astype(bf)
    wvt = wv.T  # [CENC, C] f64
    wvm = np.zeros((CENC, NPAIR * P), np.float64)
    for i, (h, t, lo, hi) in enumerate(PAIRS):
        wvm[:, i * P + (lo - P * t) : i * P + (hi - P * t)] = wvt[:, lo:hi]
    wvm = wvm.astype(bf)

    in_maps = []
    for c in range(NCORES):
        hs = hidden_states[c * BPC : (c + 1) * BPC]  # [2, S, C]
        xt = np.ascontiguousarray(hs.transpose(0, 2, 1)).astype(bf)
        enc = encoder_hidden_states[c * BPC : (c + 1) * BPC]  # [2, 77, 1024]
        et = np.empty((CENC, SENC2), np.float32)
        for b in range(BPC):
            et[:, b * SENC : (b + 1) * SENC] = enc[b].T
        in_maps.append(
            {
                "xt": xt,
                "et": et.astype(bf),
                "wqt": wqt,
                "wkt": wkt,
                "wvm": wvm,
                "wot": wot,
            }
        )
    return in_maps


def kernel(hidden_states, encoder_hidden_states, Wq, Wk, Wv, Wo, bo,
           q_down, q_up, k_down, k_up, v_down, v_up, o_down, o_up):
    nc = _get_nc()
    in_maps = make_in_maps(
        hidden_states, encoder_hidden_states, Wq, Wk, Wv, Wo,
        q_down, q_up, k_down, k_up, v_down, v_up, o_down, o_up,
    )
    res = run_bass_kernel_spmd(nc, in_maps, list(range(NCORES)))
    out = np.concatenate(
        [np.asarray(res.results[c]["out"]).astype(np.float32) for c in range(NCORES)],
        axis=0,
    )
    out = out + bo.astype(np.float32)[None, None, :]
    return out.astype(np.float32)
